# revision 19
# baseline (speedup 1.0000x reference)
"""Multi-head attention (b=2, l=2048, d_model=1024, h=16) on 8 trn2 NeuronCores.

Sharding: tensor-parallel over heads. Each core owns 2 heads: it computes the
QKV projections for its 128 channels (transposed layout), attention for its
heads, and a rank-128 partial of the output projection. The host sums the 8
partials and adds b_o (the tensor-parallel all-reduce, done at gather time).

All matmul operands are bf16 (fp32 PSUM accumulate) -- bf16 is both the
fastest PE dtype and the lowest-energy one, which matters because trn2's HAM
power manager duty-cycles the tensor engine to 4/8 when sustained utilization
exceeds its budget. The schedule therefore paces the PE *voluntarily*: tiny
pad activations dilate the ACT (exp) pacer so PE duty stays near the
sustainable level instead of bursting to 100% and tripping the 50% clamp.

Per-core schedule (single software-pipelined stream):
  warmup:  short identity-matmul burst (PE p-state) + exp-table preload,
           overlapped with the weight/x DMAs.
  proj:    QT/KT/VT [128ch, tok] = W.T @ xT per 512-token chunk; PSUM
           evacuated by ACT (Identity + per-channel bias); V re-transposed to
           key-major Vaug tiles augmented with a ones column (softmax
           denominators come from the PV matmul's 65th row). Chunks 2-7 are
           interleaved into the first attention units.
  attn:    per (batch, head, 1024-query chunk) unit: 16 k-tiles of
           scoresT = KT_h'@QT_h -> exp on ACT -> PV accumulate [65, 1024].
  fin:     per unit: evacuate PV psum, reciprocal of the Z row, broadcast
           1/Z across partitions via a tiny ones-matmul, and scale attnU ->
           attnN (pre-normalized bf16), removing all normalization from the
           output projection.
  out:     out[128tok, 1024] = attnN[:, tok-slice].T @ Wo as single
           full-128-contraction matmuls (both heads fused), interleaved into
           later units; one 256KB DMA per 128-token row block.
"""
import sys
import types

import numpy as np

D_MODEL = 1024
H = 16
DH = 64
B = 2
L = 2048
BL = B * L            # 4096 tokens
NCORES = 8
NKT = D_MODEL // 128  # 8 feature tiles
TCH = 512             # projection token chunk
NCH = BL // TCH       # 8 chunks
QC = 1024             # attention query chunk
NQC = L // QC         # 2 per batch
NKB = L // 128        # 16 k-tiles per batch
VSTRIDE = 2 * (DH + 1)  # per-k-tile Vaug columns: [V_h0 | 1 | V_h1 | 1]

# ACT pad ops per kt-iteration, per attention unit: dilates the exp pacer so
# PE duty stays under the HAM power manager's sustainable budget.
PADS = [0, 0, 0, 0, 0, 0, 0, 0]


def _register_ntff_hook():
    """Install the axon NTFF profiling hook module if the image lacks it.

    Harmless if never used; required for run_bass_kernel_spmd(trace=True)."""
    if "antenv.axon_hooks" in sys.modules:
        return
    try:
        import antenv
        mod = types.ModuleType("antenv.axon_hooks")
        holder = {}
        mod.set_axon_ntff_profile_hook = lambda h: holder.__setitem__("h", h)
        mod.get_axon_ntff_profile_hook = lambda: holder.get("h")
        sys.modules["antenv.axon_hooks"] = mod
        antenv.axon_hooks = mod
        from trn_agent_boot.trn_boot import _ntff_profile_via_ctypes
        mod.set_axon_ntff_profile_hook(
            _ntff_profile_via_ctypes("/opt/axon/libaxon_pjrt.so")
        )
    except Exception:
        pass


_NC_CACHE = {}


def _build():
    if "nc" in _NC_CACHE:
        return _NC_CACHE["nc"]
    import concourse.bacc as bacc
    import concourse.tile as tile
    import concourse.mybir as mybir

    F32 = mybir.dt.float32
    F32R = mybir.dt.float32r
    BF16 = mybir.dt.bfloat16
    AF = mybir.ActivationFunctionType
    ALU = mybir.AluOpType

    nc = bacc.Bacc("TRN2", target_bir_lowering=False, debug=False)

    xT_d = nc.dram_tensor("xT", [D_MODEL, BL], F32R, kind="ExternalInput").ap()
    wq_d = nc.dram_tensor("wq", [128, NKT * 128], F32R, kind="ExternalInput").ap()
    wk_d = nc.dram_tensor("wk", [128, NKT * 128], F32R, kind="ExternalInput").ap()
    wv_d = nc.dram_tensor("wv", [128, NKT * 128], F32R, kind="ExternalInput").ap()
    bq_d = nc.dram_tensor("bq", [128, 1], F32, kind="ExternalInput").ap()
    bk_d = nc.dram_tensor("bk", [128, 1], F32, kind="ExternalInput").ap()
    bv_d = nc.dram_tensor("bv", [128, 1], F32, kind="ExternalInput").ap()
    wo_d = nc.dram_tensor("wo", [128, D_MODEL], F32R, kind="ExternalInput").ap()
    id_d = nc.dram_tensor("ident", [128, 128], F32R, kind="ExternalInput").ap()
    out_d = nc.dram_tensor("out", [BL, D_MODEL], BF16, kind="ExternalOutput").ap()

    with tile.TileContext(nc) as tc:
        with (
            tc.tile_pool(name="weights", bufs=1) as wpool,
            tc.tile_pool(name="persist", bufs=1) as ppool,
            tc.tile_pool(name="xin", bufs=2) as xpool,
            tc.tile_pool(name="expP", bufs=3) as epool,
            tc.tile_pool(name="a65P", bufs=2) as apool,
            tc.tile_pool(name="rzP", bufs=2) as rpool,
            tc.tile_pool(name="zrowP", bufs=2) as zpool,
            tc.tile_pool(name="oout", bufs=3) as opool,
            tc.tile_pool(name="psA", bufs=2, space="PSUM") as psA,
            tc.tile_pool(name="psS", bufs=2, space="PSUM") as psS,
            tc.tile_pool(name="psPV", bufs=1, space="PSUM") as psPV,
        ):
            id_t = wpool.tile([128, 128], F32R, tag="ident")
            nc.gpsimd.dma_start(id_t[:], id_d)
            wq_t = wpool.tile([128, NKT * 128], F32R, tag="wq")
            wk_t = wpool.tile([128, NKT * 128], F32R, tag="wk")
            wv_t = wpool.tile([128, NKT * 128], F32R, tag="wv")
            bq_t = wpool.tile([128, 1], F32, tag="bq")
            bk_t = wpool.tile([128, 1], F32, tag="bk")
            bv_t = wpool.tile([128, 1], F32, tag="bv")
            wo_t = wpool.tile([128, D_MODEL], F32R, tag="wo")
            HW = NKT * 128 // 2
            nc.gpsimd.dma_start(wq_t[:, 0:HW], wq_d[:, 0:HW])
            nc.gpsimd.dma_start(wq_t[:, HW:], wq_d[:, HW:])
            nc.gpsimd.dma_start(bq_t[:], bq_d)
            nc.gpsimd.dma_start(wk_t[:, 0:HW], wk_d[:, 0:HW])
            nc.gpsimd.dma_start(wk_t[:, HW:], wk_d[:, HW:])
            nc.gpsimd.dma_start(bk_t[:], bk_d)
            nc.gpsimd.dma_start(wv_t[:, 0:HW], wv_d[:, 0:HW])
            nc.gpsimd.dma_start(wv_t[:, HW:], wv_d[:, HW:])
            nc.gpsimd.dma_start(bv_t[:], bv_d)
            nc.gpsimd.dma_start(wo_t[:, 0:HW], wo_d[:, 0:HW])
            nc.gpsimd.dma_start(wo_t[:, HW:], wo_d[:, HW:])

            QT = ppool.tile([128, BL], F32R, tag="QT")
            KT = ppool.tile([128, BL], F32R, tag="KT")
            VT = ppool.tile([128, BL], F32R, tag="VT")
            Vaug = ppool.tile([128, (BL // 128) * VSTRIDE], F32R, tag="Vaug")
            attnN = [ppool.tile([128, L], F32R, tag=f"attnN{b}",
                                name=f"attnN{b}") for b in range(B)]
            ones_t = ppool.tile([1, DH], F32R, tag="ones")
            scr = ppool.tile([1, 32], F32, tag="scr")
            scr2 = ppool.tile([1, 32], F32, tag="scr2")

            nc.vector.memset(Vaug[:].bitcast(F32), 1.0)
            nc.vector.memset(ones_t[:].bitcast(F32), 1.0)

            def emit_pad(n):
                """n tiny ACT ops: dilate the exp pacer (PE duty control)."""
                for _ in range(n):
                    nc.scalar.activation(scr2[0:1, 0:1], scr[0:1, 0:1],
                                         AF.Copy)

            # ---- warmup: PE p-state ramp + exp table preload.  The wide
            # id@Vaug fillers keep the PE visibly busy (HAM clock gate) while
            # the weight/x DMAs land; identity weights draw little power.
            def emit_filler(n):
                for i in range(n):
                    fl = psS.tile([128, QC], F32, tag="sc", name="fl")
                    nc.tensor.matmul(fl[:, 0:512], id_t[:], Vaug[:, 0:512],
                                     start=True, stop=True)

            def emit_warmup():
                wu = psS.tile([128, QC], F32, tag="sc", name="warm")
                for i in range(8):
                    nc.tensor.matmul(wu[:, 0:128], id_t[:], id_t[:],
                                     start=(i == 0), stop=(i == 7))
                nc.scalar.activation(scr[:], wu[0:1, 0:32], AF.Exp)
                emit_filler(42)

            # ---- one 512-token projection chunk (+ V re-transpose) ----
            xt_pre = {}

            def emit_chunk_dma(c, split=False):
                sl = slice(c * TCH, (c + 1) * TCH)
                xt = xpool.tile([128, NKT, TCH], F32R, tag="xchunk",
                                name="xt")
                for kt in range(NKT):
                    eng = nc.scalar if (split and kt % 2) else nc.sync
                    eng.dma_start(
                        xt[:, kt, :], xT_d[kt * 128:(kt + 1) * 128, sl]
                    )
                xt_pre[c] = xt

            def emit_chunk(c):
                sl = slice(c * TCH, (c + 1) * TCH)
                if c in xt_pre:
                    xt = xt_pre.pop(c)
                else:
                    emit_chunk_dma(c)
                    xt = xt_pre.pop(c)
                for w_t, b_t, dst in ((wq_t, bq_t, QT), (wk_t, bk_t, KT),
                                      (wv_t, bv_t, VT)):
                    ps = psA.tile([128, TCH], F32, tag="pA")
                    for kt in range(NKT):
                        nc.tensor.matmul(
                            ps[:], w_t[:, kt * 128:(kt + 1) * 128],
                            xt[:, kt, :],
                            start=(kt == 0), stop=(kt == NKT - 1),
                        )
                    # ACT evacuation: identity + per-channel bias (exp table
                    # has identity resident, so no table thrash with the exps)
                    nc.scalar.activation(dst[:, sl], ps[:], AF.Identity,
                                         bias=b_t[:, 0:1])
                tp = psA.tile([128, TCH], F32R, tag="pA", name="tp")
                for g4 in range(TCH // 128):
                    g = c * (TCH // 128) + g4
                    nc.tensor.transpose(
                        tp[:, g4 * 128:(g4 + 1) * 128],
                        VT[:, g * 128:(g + 1) * 128], id_t[:]
                    )
                for g4 in range(TCH // 128):
                    g = c * (TCH // 128) + g4
                    base = g * VSTRIDE
                    c0 = g4 * 128
                    nc.vector.tensor_copy(
                        Vaug[:, base:base + DH], tp[:, c0:c0 + DH]
                    )
                    nc.vector.tensor_copy(
                        Vaug[:, base + DH + 1:base + 2 * DH + 1],
                        tp[:, c0 + DH:c0 + 2 * DH],
                    )

            # ---- attention unit state ----
            unit_pv = {}

            def emit_kt(b, h, qc, kt, pad):
                hs = slice(h * 64, (h + 1) * 64)
                q0 = b * L + qc * QC
                if kt == 0:
                    unit_pv[(b, h, qc)] = psPV.tile([65, QC], F32, tag="pv",
                                                    name="pv")
                pv = unit_pv[(b, h, qc)]
                ksl = slice(b * L + kt * 128, b * L + (kt + 1) * 128)
                sc = psS.tile([128, QC], F32, tag="sc")
                for hf in range(QC // 512):
                    nc.tensor.matmul(
                        sc[:, hf * 512:(hf + 1) * 512],
                        KT[hs, ksl],
                        QT[hs, q0 + hf * 512:q0 + hf * 512 + 512],
                        start=True, stop=True,
                    )
                ex = epool.tile([128, QC], F32R, tag="ex")
                nc.scalar.activation(ex[:], sc[:], AF.Exp)
                emit_pad(pad)
                g = b * NKB + kt
                vb = g * VSTRIDE + h * (DH + 1)
                for hf in range(QC // 512):
                    nc.tensor.matmul(
                        pv[:, hf * 512:(hf + 1) * 512],
                        Vaug[:, vb:vb + DH + 1],
                        ex[:, hf * 512:(hf + 1) * 512],
                        start=(kt == 0), stop=(kt == NKB - 1),
                    )

            # fin part A: free the PV accumulator fast (attnU rows + Z row
            # re-based to partition 0 for the broadcast matmul). For the last
            # unit nothing reuses the PV bank, so skip the bulk copy and let
            # fin_b read psum directly -- shortens the tail critical chain.
            def emit_fin_a(b, h, qc, last=False):
                pv = unit_pv.pop((b, h, qc))
                zrow = zpool.tile([1, QC], F32R, tag="zrow")
                nc.vector.tensor_copy(zrow[:, 0:512], pv[64:65, 0:512])
                nc.vector.tensor_copy(zrow[:, 512:QC], pv[64:65, 512:QC])
                if last:
                    return (b, h, qc, pv[0:DH, :], zrow)
                a64 = apool.tile([DH, QC], F32R, tag="a64")
                nc.vector.tensor_copy(a64[:], pv[0:DH, :])
                return (b, h, qc, a64, zrow)

            # fin part B (emitted ~2 kt-iters later): broadcast Z across 64
            # partitions via ones-matmul, take its fast reciprocal with all
            # partitions in parallel, then attnN = attnU * (1/Z). (An exact
            # 1-partition InstReciprocal on the Z row costs 6.4ns/element and
            # stalls the PE at every unit boundary.)
            def emit_fin_b(st):
                b, h, qc, a64, zrow = st
                zr = rpool.tile([DH, QC], F32R, tag="zr")
                for half in range(QC // 512):
                    hsl = slice(half * 512, (half + 1) * 512)
                    zb = psA.tile([128, TCH], F32, tag="pA", name="zb")
                    nc.tensor.matmul(
                        zb[0:DH, :], ones_t[0:1, :], zrow[0:1, hsl],
                        start=True, stop=True,
                    )
                    nc.vector.reciprocal_approx_fast(
                        zr[:, hsl].bitcast(F32), zb[0:DH, :])
                    dst = attnN[b][h * 64:(h + 1) * 64,
                                   qc * QC + half * 512:
                                   qc * QC + (half + 1) * 512]
                    nc.vector.tensor_tensor(
                        dst, a64[:, hsl], zr[:, hsl], op=ALU.mult,
                    )

            # ---- output projection unit: 128 tokens x 1024, both heads ----
            out_tiles = {}

            def emit_cunit(b, rc, oc, tail=False):
                rsl = slice(b * L + rc * 128, b * L + (rc + 1) * 128)
                lsl = slice(rc * 128, (rc + 1) * 128)
                if tail and (rc + oc) % 2 == 1:
                    # scores are done: borrow the free psS banks so tail
                    # matmuls never wait on evacuation latency
                    ps = psS.tile([128, QC], F32, tag="sc", name="cps")[:, 0:512]
                else:
                    ps = psA.tile([128, TCH], F32, tag="pA", name="cps")[:]
                nc.tensor.matmul(ps, attnN[b][:, lsl],
                                 wo_t[:, oc * 512:(oc + 1) * 512],
                                 start=True, stop=True)
                if oc == 0:
                    out_tiles[(b, rc)] = opool.tile([128, D_MODEL], BF16,
                                                    tag="ot", name="ot")
                ot = out_tiles[(b, rc)]
                osl = slice(oc * 512, (oc + 1) * 512)
                if tail and oc == 1:
                    # after the last exp ACT is free: split tail evacuations
                    nc.scalar.activation(ot[:, osl], ps, AF.Copy)
                else:
                    nc.vector.tensor_copy(ot[:, osl], ps)
                if tail:
                    nc.sync.dma_start(out_d[rsl, osl], ot[:, osl])
                    if oc == 1:
                        out_tiles.pop((b, rc))
                elif oc == 1:
                    nc.sync.dma_start(out_d[rsl, :], out_tiles.pop((b, rc))[:])

            # ---- schedule assembly ----
            # units ordered so c_units unlock early: per batch h0qc0, h1qc0,
            # h0qc1, h1qc1 -> tokens [0:1024] of both heads done after unit 2.
            units = [(b, h, qc) for b in range(B) for qc in range(NQC)
                     for h in range(2)]
            # proj chunks 2..7 injected into units 0..3 at key-availability-
            # safe positions.
            chunk_sched = {0: {2: 2, 7: 3}, 1: {2: 4}, 2: {2: 5, 9: 6},
                           3: {2: 7}}
            # c_units rc-blocks injected per unit index (8 c_units each over
            # 16 kt-iters), unlocked by fin of two units prior.
            cunit_sched = {
                2: [(0, rc) for rc in range(0, 4)],
                3: [(0, rc) for rc in range(4, 8)],
                4: [(0, rc) for rc in range(8, 12)],
                5: [(0, rc) for rc in range(12, 16)],
                6: [(1, rc) for rc in range(0, 8)],
            }

            emit_chunk_dma(0, split=True)
            emit_chunk_dma(1)
            emit_warmup()
            emit_chunk(0)
            emit_chunk(1)
            pending_fin = None
            for ui, (b, h, qc) in enumerate(units):
                cq = [(cb, rc, oc) for (cb, rc) in cunit_sched.get(ui, [])
                      for oc in range(2)]
                for kt in range(NKB):
                    emit_kt(b, h, qc, kt, PADS[ui])
                    if kt == 1 and pending_fin is not None:
                        emit_fin_b(pending_fin)
                        pending_fin = None
                    if kt in chunk_sched.get(ui, {}):
                        emit_chunk(chunk_sched[ui][kt])
                    if kt > 1 and cq:
                        if kt % 2 == 1 or len(cq) > NKB - 1 - kt:
                            emit_cunit(*cq.pop(0))
                while cq:
                    emit_cunit(*cq.pop(0))
                pending_fin = emit_fin_a(b, h, qc, last=(ui == len(units) - 1))
            # keep the clock gate shut across the final fin chain
            emit_filler(12)
            emit_fin_b(pending_fin)
            # tail: last half of batch-1 output rows
            for rc in range(8, 16):
                for oc in range(2):
                    emit_cunit(1, rc, oc, tail=True)

    nc.compile()
    _NC_CACHE["nc"] = nc
    return nc


def _shard_inputs(x, W_qkv, b_qkv, W_o):
    import ml_dtypes
    BF = ml_dtypes.bfloat16
    xT = np.ascontiguousarray(
        x.reshape(BL, D_MODEL).T, dtype=np.float32
    )
    ident = np.eye(128, dtype=np.float32)

    def lhsT_layout(w):
        # [D_MODEL, 128] -> [128, NKT*128] with [p, kt*128+ch] = w[kt*128+p, ch]
        return np.ascontiguousarray(
            w.reshape(NKT, 128, 128).transpose(1, 0, 2).reshape(128, NKT * 128),
            dtype=np.float32,
        )

    in_maps = []
    for c in range(NCORES):
        cs = slice(c * 128, (c + 1) * 128)
        wq = W_qkv[:, cs] * 0.125
        wk = W_qkv[:, D_MODEL:][:, cs]
        wv = W_qkv[:, 2 * D_MODEL:][:, cs]
        in_maps.append({
            "xT": xT,
            "wq": lhsT_layout(wq), "wk": lhsT_layout(wk), "wv": lhsT_layout(wv),
            "bq": np.ascontiguousarray(
                b_qkv[cs] * 0.125, dtype=np.float32).reshape(128, 1),
            "bk": np.ascontiguousarray(
                b_qkv[D_MODEL:][cs], dtype=np.float32).reshape(128, 1),
            "bv": np.ascontiguousarray(
                b_qkv[2 * D_MODEL:][cs], dtype=np.float32).reshape(128, 1),
            "wo": np.ascontiguousarray(W_o[cs, :], dtype=np.float32),
            "ident": ident,
        })
    return in_maps


def _run(inputs, trace=False, tmpdir=None):
    from concourse.bass_utils import run_bass_kernel_spmd

    _register_ntff_hook()
    nc = _build()
    in_maps = _shard_inputs(
        np.asarray(inputs["x"], dtype=np.float32),
        np.asarray(inputs["W_qkv"], dtype=np.float32),
        np.asarray(inputs["b_qkv"], dtype=np.float32),
        np.asarray(inputs["W_o"], dtype=np.float32),
    )
    res = run_bass_kernel_spmd(nc, in_maps, core_ids=list(range(NCORES)),
                               trace=trace, tmpdir=tmpdir)
    partial = np.zeros((BL, D_MODEL), dtype=np.float64)
    for c in range(NCORES):
        partial += np.asarray(res.results[c]["out"]).astype(np.float64)
    out = (partial + np.asarray(inputs["b_o"], dtype=np.float64)).astype(np.float32)
    return out.reshape(B, L, D_MODEL), res


def kernel(**inputs) -> np.ndarray:
    out, _ = _run(inputs, trace=False)
    return out


# revision 20
# speedup vs baseline: 1.0975x; 1.0975x over previous
"""Multi-head attention (b=2, l=2048, d_model=1024, h=16) on 8 trn2 NeuronCores.

Sharding: tensor-parallel over heads. Each core owns 2 heads: it computes the
QKV projections for its 128 channels (transposed layout), attention for its
heads, and a rank-128 partial of the output projection. The host sums the 8
partials and adds b_o (the tensor-parallel all-reduce, done at gather time).

All matmul operands are bf16 (fp32 PSUM accumulate) -- bf16 is both the
fastest PE dtype and the lowest-energy one, which matters because trn2's HAM
power manager duty-cycles the tensor engine to 4/8 when sustained utilization
exceeds its budget. The schedule therefore paces the PE *voluntarily*: tiny
pad activations dilate the ACT (exp) pacer so PE duty stays near the
sustainable level instead of bursting to 100% and tripping the 50% clamp.

Per-core schedule (single software-pipelined stream):
  warmup:  short identity-matmul burst (PE p-state) + exp-table preload,
           overlapped with the weight/x DMAs.
  proj:    QT/KT/VT [128ch, tok] = W.T @ xT per 512-token chunk; PSUM
           evacuated by ACT (Identity + per-channel bias); V re-transposed to
           key-major Vaug tiles augmented with a ones column (softmax
           denominators come from the PV matmul's 65th row). Chunks 2-7 are
           interleaved into the first attention units.
  attn:    per (batch, head, 1024-query chunk) unit: 16 k-tiles of
           scoresT = KT_h'@QT_h -> exp on ACT -> PV accumulate [65, 1024].
  fin:     per unit: evacuate PV psum, reciprocal of the Z row, broadcast
           1/Z across partitions via a tiny ones-matmul, and scale attnU ->
           attnN (pre-normalized bf16), removing all normalization from the
           output projection.
  out:     out[128tok, 1024] = attnN[:, tok-slice].T @ Wo as single
           full-128-contraction matmuls (both heads fused), interleaved into
           later units; one 256KB DMA per 128-token row block.
"""
import sys
import types

import numpy as np

D_MODEL = 1024
H = 16
DH = 64
B = 2
L = 2048
BL = B * L            # 4096 tokens
NCORES = 8
NKT = D_MODEL // 128  # 8 feature tiles
TCH = 512             # projection token chunk
NCH = BL // TCH       # 8 chunks
QC = 1024             # attention query chunk
NQC = L // QC         # 2 per batch
NKB = L // 128        # 16 k-tiles per batch
VSTRIDE = 2 * (DH + 1)  # per-k-tile Vaug columns: [V_h0 | 1 | V_h1 | 1]

# ACT pad ops per kt-iteration, per attention unit: dilates the exp pacer so
# PE duty stays under the HAM power manager's sustainable budget.
PADS = [0, 0, 0, 0, 0, 0, 0, 0]


def _register_ntff_hook():
    """Install the axon NTFF profiling hook module if the image lacks it.

    Harmless if never used; required for run_bass_kernel_spmd(trace=True)."""
    if "antenv.axon_hooks" in sys.modules:
        return
    try:
        import antenv
        mod = types.ModuleType("antenv.axon_hooks")
        holder = {}
        mod.set_axon_ntff_profile_hook = lambda h: holder.__setitem__("h", h)
        mod.get_axon_ntff_profile_hook = lambda: holder.get("h")
        sys.modules["antenv.axon_hooks"] = mod
        antenv.axon_hooks = mod
        from trn_agent_boot.trn_boot import _ntff_profile_via_ctypes
        mod.set_axon_ntff_profile_hook(
            _ntff_profile_via_ctypes("/opt/axon/libaxon_pjrt.so")
        )
    except Exception:
        pass


_NC_CACHE = {}


def _build():
    if "nc" in _NC_CACHE:
        return _NC_CACHE["nc"]
    import concourse.bacc as bacc
    import concourse.tile as tile
    import concourse.mybir as mybir

    F32 = mybir.dt.float32
    F32R = mybir.dt.float32r
    BF16 = mybir.dt.bfloat16
    AF = mybir.ActivationFunctionType
    ALU = mybir.AluOpType

    nc = bacc.Bacc("TRN2", target_bir_lowering=False, debug=False)

    xT_d = nc.dram_tensor("xT", [D_MODEL, BL], BF16, kind="ExternalInput").ap()
    wq_d = nc.dram_tensor("wq", [128, NKT * 128], BF16, kind="ExternalInput").ap()
    wk_d = nc.dram_tensor("wk", [128, NKT * 128], BF16, kind="ExternalInput").ap()
    wv_d = nc.dram_tensor("wv", [128, NKT * 128], BF16, kind="ExternalInput").ap()
    bq_d = nc.dram_tensor("bq", [128, 1], F32, kind="ExternalInput").ap()
    bk_d = nc.dram_tensor("bk", [128, 1], F32, kind="ExternalInput").ap()
    bv_d = nc.dram_tensor("bv", [128, 1], F32, kind="ExternalInput").ap()
    wo_d = nc.dram_tensor("wo", [128, D_MODEL], BF16, kind="ExternalInput").ap()
    id_d = nc.dram_tensor("ident", [128, 128], BF16, kind="ExternalInput").ap()
    out_d = nc.dram_tensor("out", [BL, D_MODEL], BF16, kind="ExternalOutput").ap()

    with tile.TileContext(nc) as tc:
        with (
            tc.tile_pool(name="weights", bufs=1) as wpool,
            tc.tile_pool(name="persist", bufs=1) as ppool,
            tc.tile_pool(name="xin", bufs=2) as xpool,
            tc.tile_pool(name="expP", bufs=3) as epool,
            tc.tile_pool(name="a65P", bufs=2) as apool,
            tc.tile_pool(name="rzP", bufs=2) as rpool,
            tc.tile_pool(name="zrowP", bufs=2) as zpool,
            tc.tile_pool(name="oout", bufs=3) as opool,
            tc.tile_pool(name="psA", bufs=2, space="PSUM") as psA,
            tc.tile_pool(name="psS", bufs=2, space="PSUM") as psS,
            tc.tile_pool(name="psPV", bufs=1, space="PSUM") as psPV,
        ):
            id_t = wpool.tile([128, 128], BF16, tag="ident")
            nc.gpsimd.dma_start(id_t[:], id_d)
            wq_t = wpool.tile([128, NKT * 128], BF16, tag="wq")
            wk_t = wpool.tile([128, NKT * 128], BF16, tag="wk")
            wv_t = wpool.tile([128, NKT * 128], BF16, tag="wv")
            bq_t = wpool.tile([128, 1], F32, tag="bq")
            bk_t = wpool.tile([128, 1], F32, tag="bk")
            bv_t = wpool.tile([128, 1], F32, tag="bv")
            wo_t = wpool.tile([128, D_MODEL], BF16, tag="wo")
            HW = NKT * 128 // 2
            nc.gpsimd.dma_start(wq_t[:, 0:HW], wq_d[:, 0:HW])
            nc.scalar.dma_start(wq_t[:, HW:], wq_d[:, HW:])
            nc.gpsimd.dma_start(bq_t[:], bq_d)
            nc.gpsimd.dma_start(wk_t[:, 0:HW], wk_d[:, 0:HW])
            nc.scalar.dma_start(wk_t[:, HW:], wk_d[:, HW:])
            nc.gpsimd.dma_start(bk_t[:], bk_d)
            nc.gpsimd.dma_start(wv_t[:, 0:HW], wv_d[:, 0:HW])
            nc.scalar.dma_start(wv_t[:, HW:], wv_d[:, HW:])
            nc.gpsimd.dma_start(bv_t[:], bv_d)
            nc.gpsimd.dma_start(wo_t[:, 0:HW], wo_d[:, 0:HW])
            nc.gpsimd.dma_start(wo_t[:, HW:], wo_d[:, HW:])

            QT = ppool.tile([128, BL], BF16, tag="QT")
            KT = ppool.tile([128, BL], BF16, tag="KT")
            VT = ppool.tile([128, BL], BF16, tag="VT")
            Vaug = ppool.tile([128, (BL // 128) * VSTRIDE], BF16, tag="Vaug")
            attnN = [ppool.tile([128, L], BF16, tag=f"attnN{b}",
                                name=f"attnN{b}") for b in range(B)]
            ones_t = ppool.tile([1, DH], F32R, tag="ones")
            scr = ppool.tile([1, 32], F32, tag="scr")
            scr2 = ppool.tile([1, 32], F32, tag="scr2")

            nc.vector.memset(Vaug[:], 1.0)
            nc.vector.memset(ones_t[:].bitcast(F32), 1.0)

            def emit_pad(n):
                """n tiny ACT ops: dilate the exp pacer (PE duty control)."""
                for _ in range(n):
                    nc.scalar.activation(scr2[0:1, 0:1], scr[0:1, 0:1],
                                         AF.Copy)

            # ---- warmup: PE p-state ramp + exp table preload.  The wide
            # id@Vaug fillers keep the PE visibly busy (HAM clock gate) while
            # the weight/x DMAs land; identity weights draw little power.
            def emit_filler(n):
                for i in range(n):
                    fl = psS.tile([128, QC], F32, tag="sc", name="fl")
                    nc.tensor.matmul(fl[:, 0:512], id_t[:], Vaug[:, 0:512],
                                     start=True, stop=True)

            def emit_warmup():
                wu = psS.tile([128, QC], F32, tag="sc", name="warm")
                for i in range(8):
                    nc.tensor.matmul(wu[:, 0:128], id_t[:], id_t[:],
                                     start=(i == 0), stop=(i == 7))
                nc.scalar.activation(scr[:], wu[0:1, 0:32], AF.Exp)
                emit_filler(58)

            # ---- one 512-token projection chunk (+ V re-transpose) ----
            xt_pre = {}

            def emit_chunk_dma(c, split=False):
                sl = slice(c * TCH, (c + 1) * TCH)
                xt = xpool.tile([128, NKT, TCH], BF16, tag="xchunk",
                                name="xt")
                for kt in range(NKT):
                    nc.sync.dma_start(
                        xt[:, kt, :], xT_d[kt * 128:(kt + 1) * 128, sl]
                    )
                xt_pre[c] = xt

            def emit_chunk(c):
                sl = slice(c * TCH, (c + 1) * TCH)
                if c in xt_pre:
                    xt = xt_pre.pop(c)
                else:
                    emit_chunk_dma(c)
                    xt = xt_pre.pop(c)
                for w_t, b_t, dst in ((wq_t, bq_t, QT), (wk_t, bk_t, KT),
                                      (wv_t, bv_t, VT)):
                    ps = psA.tile([128, TCH], F32, tag="pA")
                    for kt in range(NKT):
                        nc.tensor.matmul(
                            ps[:], w_t[:, kt * 128:(kt + 1) * 128],
                            xt[:, kt, :],
                            start=(kt == 0), stop=(kt == NKT - 1),
                        )
                    # ACT evacuation: identity + per-channel bias (exp table
                    # has identity resident, so no table thrash with the exps)
                    nc.scalar.activation(dst[:, sl], ps[:], AF.Identity,
                                         bias=b_t[:, 0:1])
                tp = psA.tile([128, TCH], BF16, tag="pA", name="tp")
                for g4 in range(TCH // 128):
                    g = c * (TCH // 128) + g4
                    nc.tensor.transpose(
                        tp[:, g4 * 128:(g4 + 1) * 128],
                        VT[:, g * 128:(g + 1) * 128], id_t[:]
                    )
                for g4 in range(TCH // 128):
                    g = c * (TCH // 128) + g4
                    base = g * VSTRIDE
                    c0 = g4 * 128
                    nc.vector.tensor_copy(
                        Vaug[:, base:base + DH], tp[:, c0:c0 + DH]
                    )
                    nc.vector.tensor_copy(
                        Vaug[:, base + DH + 1:base + 2 * DH + 1],
                        tp[:, c0 + DH:c0 + 2 * DH],
                    )

            # ---- attention unit state ----
            unit_pv = {}

            def emit_kt(b, h, qc, kt, pad):
                hs = slice(h * 64, (h + 1) * 64)
                q0 = b * L + qc * QC
                if kt == 0:
                    unit_pv[(b, h, qc)] = psPV.tile([65, QC], F32, tag="pv",
                                                    name="pv")
                pv = unit_pv[(b, h, qc)]
                ksl = slice(b * L + kt * 128, b * L + (kt + 1) * 128)
                sc = psS.tile([128, QC], F32, tag="sc")
                for hf in range(QC // 512):
                    nc.tensor.matmul(
                        sc[:, hf * 512:(hf + 1) * 512],
                        KT[hs, ksl],
                        QT[hs, q0 + hf * 512:q0 + hf * 512 + 512],
                        start=True, stop=True,
                    )
                ex = epool.tile([128, QC], BF16, tag="ex")
                nc.scalar.activation(ex[:], sc[:], AF.Exp)
                emit_pad(pad)
                g = b * NKB + kt
                vb = g * VSTRIDE + h * (DH + 1)
                for hf in range(QC // 512):
                    nc.tensor.matmul(
                        pv[:, hf * 512:(hf + 1) * 512],
                        Vaug[:, vb:vb + DH + 1],
                        ex[:, hf * 512:(hf + 1) * 512],
                        start=(kt == 0), stop=(kt == NKB - 1),
                    )

            # fin part A: free the PV accumulator fast (attnU rows + Z row
            # re-based to partition 0 for the broadcast matmul). For the last
            # unit nothing reuses the PV bank, so skip the bulk copy and let
            # fin_b read psum directly -- shortens the tail critical chain.
            def emit_fin_a(b, h, qc, last=False):
                pv = unit_pv.pop((b, h, qc))
                zrow = zpool.tile([1, QC], F32R, tag="zrow")
                nc.vector.tensor_copy(zrow[:, 0:512], pv[64:65, 0:512])
                nc.vector.tensor_copy(zrow[:, 512:QC], pv[64:65, 512:QC])
                if last:
                    return (b, h, qc, pv[0:DH, :], zrow)
                a64 = apool.tile([DH, QC], F32R, tag="a64")
                nc.vector.tensor_copy(a64[:], pv[0:DH, :])
                return (b, h, qc, a64, zrow)

            # fin part B (emitted ~2 kt-iters later): broadcast Z across 64
            # partitions via ones-matmul, take its fast reciprocal with all
            # partitions in parallel, then attnN = attnU * (1/Z). (An exact
            # 1-partition InstReciprocal on the Z row costs 6.4ns/element and
            # stalls the PE at every unit boundary.)
            def emit_fin_b(st):
                b, h, qc, a64, zrow = st
                zr = rpool.tile([DH, QC], F32R, tag="zr")
                for half in range(QC // 512):
                    hsl = slice(half * 512, (half + 1) * 512)
                    zb = psA.tile([128, TCH], F32, tag="pA", name="zb")
                    nc.tensor.matmul(
                        zb[0:DH, :], ones_t[0:1, :], zrow[0:1, hsl],
                        start=True, stop=True,
                    )
                    nc.vector.reciprocal_approx_fast(
                        zr[:, hsl].bitcast(F32), zb[0:DH, :])
                    dst = attnN[b][h * 64:(h + 1) * 64,
                                   qc * QC + half * 512:
                                   qc * QC + (half + 1) * 512]
                    nc.vector.tensor_tensor(
                        dst, a64[:, hsl], zr[:, hsl], op=ALU.mult,
                    )

            # ---- output projection unit: 128 tokens x 1024, both heads ----
            out_tiles = {}

            def emit_cunit(b, rc, oc, tail=False):
                rsl = slice(b * L + rc * 128, b * L + (rc + 1) * 128)
                lsl = slice(rc * 128, (rc + 1) * 128)
                if tail and (rc + oc) % 2 == 1:
                    # scores are done: borrow the free psS banks so tail
                    # matmuls never wait on evacuation latency
                    ps = psS.tile([128, QC], F32, tag="sc", name="cps")[:, 0:512]
                else:
                    ps = psA.tile([128, TCH], F32, tag="pA", name="cps")[:]
                nc.tensor.matmul(ps, attnN[b][:, lsl],
                                 wo_t[:, oc * 512:(oc + 1) * 512],
                                 start=True, stop=True)
                if oc == 0:
                    out_tiles[(b, rc)] = opool.tile([128, D_MODEL], BF16,
                                                    tag="ot", name="ot")
                ot = out_tiles[(b, rc)]
                osl = slice(oc * 512, (oc + 1) * 512)
                if tail and oc == 1:
                    # after the last exp ACT is free: split tail evacuations
                    nc.scalar.activation(ot[:, osl], ps, AF.Copy)
                else:
                    nc.vector.tensor_copy(ot[:, osl], ps)
                if tail:
                    nc.sync.dma_start(out_d[rsl, osl], ot[:, osl])
                    if oc == 1:
                        out_tiles.pop((b, rc))
                elif oc == 1:
                    nc.sync.dma_start(out_d[rsl, :], out_tiles.pop((b, rc))[:])

            # ---- schedule assembly ----
            # units ordered so c_units unlock early: per batch h0qc0, h1qc0,
            # h0qc1, h1qc1 -> tokens [0:1024] of both heads done after unit 2.
            units = [(b, h, qc) for b in range(B) for qc in range(NQC)
                     for h in range(2)]
            # proj chunks 2..7 injected into units 0..3 at key-availability-
            # safe positions.
            chunk_sched = {0: {2: 2, 7: 3}, 1: {2: 4}, 2: {2: 5, 9: 6},
                           3: {2: 7}}
            # c_units rc-blocks injected per unit index (8 c_units each over
            # 16 kt-iters), unlocked by fin of two units prior.
            cunit_sched = {
                2: [(0, rc) for rc in range(0, 4)],
                3: [(0, rc) for rc in range(4, 8)],
                4: [(0, rc) for rc in range(8, 12)],
                5: [(0, rc) for rc in range(12, 16)],
                6: [(1, rc) for rc in range(0, 8)],
            }

            emit_chunk_dma(0, split=True)
            emit_chunk_dma(1)
            emit_warmup()
            emit_chunk(0)
            emit_chunk(1)
            pending_fin = None
            for ui, (b, h, qc) in enumerate(units):
                cq = [(cb, rc, oc) for (cb, rc) in cunit_sched.get(ui, [])
                      for oc in range(2)]
                for kt in range(NKB):
                    emit_kt(b, h, qc, kt, PADS[ui])
                    if kt == 1 and pending_fin is not None:
                        emit_fin_b(pending_fin)
                        pending_fin = None
                    if kt in chunk_sched.get(ui, {}):
                        emit_chunk(chunk_sched[ui][kt])
                    if kt > 1 and cq:
                        if kt % 2 == 1 or len(cq) > NKB - 1 - kt:
                            emit_cunit(*cq.pop(0))
                while cq:
                    emit_cunit(*cq.pop(0))
                pending_fin = emit_fin_a(b, h, qc, last=(ui == len(units) - 1))
            # keep the clock gate shut across the final fin chain
            emit_filler(12)
            emit_fin_b(pending_fin)
            # tail: last half of batch-1 output rows
            for rc in range(8, 16):
                for oc in range(2):
                    emit_cunit(1, rc, oc, tail=True)

    nc.compile()
    _NC_CACHE["nc"] = nc
    return nc


def _shard_inputs(x, W_qkv, b_qkv, W_o):
    import ml_dtypes
    BF = ml_dtypes.bfloat16
    xT = np.ascontiguousarray(
        x.reshape(BL, D_MODEL).T, dtype=np.float32
    ).astype(BF)
    ident = np.eye(128, dtype=np.float32).astype(BF)

    def lhsT_layout(w):
        # [D_MODEL, 128] -> [128, NKT*128] with [p, kt*128+ch] = w[kt*128+p, ch]
        return np.ascontiguousarray(
            w.reshape(NKT, 128, 128).transpose(1, 0, 2).reshape(128, NKT * 128),
            dtype=np.float32,
        ).astype(BF)

    in_maps = []
    for c in range(NCORES):
        cs = slice(c * 128, (c + 1) * 128)
        wq = W_qkv[:, cs] * 0.125
        wk = W_qkv[:, D_MODEL:][:, cs]
        wv = W_qkv[:, 2 * D_MODEL:][:, cs]
        in_maps.append({
            "xT": xT,
            "wq": lhsT_layout(wq), "wk": lhsT_layout(wk), "wv": lhsT_layout(wv),
            "bq": np.ascontiguousarray(
                b_qkv[cs] * 0.125, dtype=np.float32).reshape(128, 1),
            "bk": np.ascontiguousarray(
                b_qkv[D_MODEL:][cs], dtype=np.float32).reshape(128, 1),
            "bv": np.ascontiguousarray(
                b_qkv[2 * D_MODEL:][cs], dtype=np.float32).reshape(128, 1),
            "wo": np.ascontiguousarray(
                W_o[cs, :], dtype=np.float32).astype(BF),
            "ident": ident,
        })
    return in_maps


def _run(inputs, trace=False, tmpdir=None):
    from concourse.bass_utils import run_bass_kernel_spmd

    _register_ntff_hook()
    nc = _build()
    in_maps = _shard_inputs(
        np.asarray(inputs["x"], dtype=np.float32),
        np.asarray(inputs["W_qkv"], dtype=np.float32),
        np.asarray(inputs["b_qkv"], dtype=np.float32),
        np.asarray(inputs["W_o"], dtype=np.float32),
    )
    res = run_bass_kernel_spmd(nc, in_maps, core_ids=list(range(NCORES)),
                               trace=trace, tmpdir=tmpdir)
    partial = np.zeros((BL, D_MODEL), dtype=np.float64)
    for c in range(NCORES):
        partial += np.asarray(res.results[c]["out"]).astype(np.float64)
    out = (partial + np.asarray(inputs["b_o"], dtype=np.float64)).astype(np.float32)
    return out.reshape(B, L, D_MODEL), res


def kernel(**inputs) -> np.ndarray:
    out, _ = _run(inputs, trace=False)
    return out
